# revision 4
# baseline (speedup 1.0000x reference)
"""MoE (top-2 of 8 experts) Trainium2 kernel.

Strategy: expert-parallel across 8 NeuronCores. The router (8192x1024 @
1024x8 + top-k) is tiny, so it runs on host in float64 (verified to
reproduce the fp32 reference ranking). Each core gets the tokens routed
to its expert (capacity 2304 >= observed max 2203), runs the dense
2-layer FFN in transposed layout with float32r matmuls (full PE rate at
N>=256, ~1.5e-4 matmul error) and exact-erf Gelu on ScalarE, and the
host scatter-adds the two expert contributions per token.
"""

import sys

sys.path.insert(0, "/opt/trn_rl_repo")

import math

import numpy as np

_B, _SEQ, _E, _H, _NE = 4, 2048, 1024, 1536, 8
_T = _B * _SEQ
_CAP = 2304  # per-expert token capacity (multiple of 128; >= max count 2203)
_COLT = [(0, 512), (512, 512), (1024, 512), (1536, 512), (2048, 256)]
_NCORES = 8
_P = 128

_nc_cache: dict = {}


def _build_nc(inv_k: float, repeat: int = 1):
    from contextlib import ExitStack

    import concourse.tile as tile
    from concourse import bacc, mybir

    f32 = mybir.dt.float32
    f32r = mybir.dt.float32r
    KO1 = _E // _P   # 8  k-tiles for layer-1 contraction
    HT = _H // _P    # 12 h-tiles (layer-1 out / layer-2 contraction)
    ET = _E // _P    # 8  e-tiles (layer-2 out)

    nc = bacc.Bacc("TRN2", target_bir_lowering=False, debug=False)
    xt_d = nc.dram_tensor("xt", [_E, _CAP], f32r, kind="ExternalInput").ap()
    w1_d = nc.dram_tensor("w1", [_E, _H], f32r, kind="ExternalInput").ap()
    w2_d = nc.dram_tensor("w2", [_H, _E], f32r, kind="ExternalInput").ap()
    b1_d = nc.dram_tensor("b1p", [_P, HT], f32, kind="ExternalInput").ap()
    b2_d = nc.dram_tensor("b2p", [_P, ET], f32, kind="ExternalInput").ap()
    yt_d = nc.dram_tensor("yt", [_E, _CAP], f32, kind="ExternalOutput").ap()

    with tile.TileContext(nc) as tc:
        with ExitStack() as ctx:
            wpool = ctx.enter_context(tc.tile_pool(name="w", bufs=1))
            cpool = ctx.enter_context(tc.tile_pool(name="c", bufs=1))
            xpool = ctx.enter_context(tc.tile_pool(name="x", bufs=2))
            hpool = ctx.enter_context(tc.tile_pool(name="h", bufs=2))
            ypool = ctx.enter_context(tc.tile_pool(name="y", bufs=4))
            ps1 = ctx.enter_context(tc.tile_pool(name="ps1", bufs=4, space="PSUM"))
            ps2 = ctx.enter_context(tc.tile_pool(name="ps2", bufs=4, space="PSUM"))

            # resident weights: [128, ko*H] / [128, hi*E] blocks
            w1_sb = wpool.tile([_P, KO1 * _H], f32r, tag="w1")
            for ko in range(KO1):
                nc.gpsimd.dma_start(
                    w1_sb[:, ko * _H : (ko + 1) * _H],
                    w1_d[ko * _P : (ko + 1) * _P, :],
                )
            w2_sb = wpool.tile([_P, HT * _E], f32r, tag="w2")
            for hi in range(HT):
                nc.gpsimd.dma_start(
                    w2_sb[:, hi * _E : (hi + 1) * _E],
                    w2_d[hi * _P : (hi + 1) * _P, :],
                )
            b1_sb = cpool.tile([_P, HT], f32, tag="b1")
            nc.gpsimd.dma_start(b1_sb[:], b1_d[:, :])
            b2_sb = cpool.tile([_P, ET], f32, tag="b2")
            nc.gpsimd.dma_start(b2_sb[:], b2_d[:, :])

            for _ in range(repeat):
                for c0, nt in _COLT:
                    xt_sb = xpool.tile([_P, KO1 * nt], f32r, tag="xt")
                    for ko in range(KO1):
                        nc.gpsimd.dma_start(
                            xt_sb[:, ko * nt : (ko + 1) * nt],
                            xt_d[ko * _P : (ko + 1) * _P, c0 : c0 + nt],
                        )
                    # h.T = gelu(W1.T @ x.T + b1)
                    h_sb = hpool.tile([_P, HT * nt], f32r, tag="h")
                    for hi in range(HT):
                        acc = ps1.tile([_P, nt], f32, tag="ps1")
                        for ko in range(KO1):
                            nc.tensor.matmul(
                                acc[:],
                                w1_sb[:, ko * _H + hi * _P : ko * _H + (hi + 1) * _P],
                                xt_sb[:, ko * nt : (ko + 1) * nt],
                                start=(ko == 0),
                                stop=(ko == KO1 - 1),
                            )
                        nc.scalar.activation(
                            h_sb[:, hi * nt : (hi + 1) * nt],
                            acc[:],
                            mybir.ActivationFunctionType.Gelu,
                            bias=b1_sb[:, hi : hi + 1],
                            scale=1.0,
                        )
                    # y.T = (W2.T @ h.T + b2) * inv_k
                    for ei in range(ET):
                        acc2 = ps2.tile([_P, nt], f32, tag="ps2")
                        for hi in range(HT):
                            nc.tensor.matmul(
                                acc2[:],
                                w2_sb[:, hi * _E + ei * _P : hi * _E + (ei + 1) * _P],
                                h_sb[:, hi * nt : (hi + 1) * nt],
                                start=(hi == 0),
                                stop=(hi == HT - 1),
                            )
                        y_sb = ypool.tile([_P, nt], f32, tag="y")
                        nc.scalar.activation(
                            y_sb[:],
                            acc2[:],
                            mybir.ActivationFunctionType.Identity,
                            bias=b2_sb[:, ei : ei + 1],
                            scale=inv_k,
                        )
                        nc.gpsimd.dma_start(
                            yt_d[ei * _P : (ei + 1) * _P, c0 : c0 + nt], y_sb[:]
                        )
    nc.compile()
    return nc


def _make_runner(nc, n_cores):
    """Persistent-jit SPMD runner (modeled on bass2jax.run_bass_via_pjrt)."""
    import jax
    import numpy as _np
    from jax.sharding import Mesh, PartitionSpec
    from jax.experimental.shard_map import shard_map

    from concourse import mybir
    from concourse.bass2jax import (
        _bass_exec_p,
        install_neuronx_cc_hook,
        partition_id_tensor,
    )

    install_neuronx_cc_hook()

    partition_name = nc.partition_id_tensor.name if nc.partition_id_tensor else None
    in_names: list = []
    out_names: list = []
    out_avals: list = []
    zero_outs: list = []
    for alloc in nc.m.functions[0].allocations:
        if not isinstance(alloc, mybir.MemoryLocationSet):
            continue
        name = alloc.memorylocations[0].name
        if alloc.kind == "ExternalInput":
            if name != partition_name:
                in_names.append(name)
        elif alloc.kind == "ExternalOutput":
            shape = tuple(alloc.tensor_shape)
            dtype = mybir.dt.np(alloc.dtype)
            out_names.append(name)
            out_avals.append(jax.core.ShapedArray(shape, dtype))
            zero_outs.append(_np.zeros(shape, dtype))
    n_params = len(in_names)
    n_outs = len(out_avals)
    all_in_names = in_names + out_names
    if partition_name is not None:
        all_in_names = all_in_names + [partition_name]

    def _body(*args):
        operands = list(args)
        if partition_name is not None:
            operands.append(partition_id_tensor())
        outs = _bass_exec_p.bind(
            *operands,
            out_avals=tuple(out_avals),
            in_names=tuple(all_in_names),
            out_names=tuple(out_names),
            lowering_input_output_aliases=(),
            sim_require_finite=True,
            sim_require_nnan=True,
            nc=nc,
        )
        return tuple(outs)

    devices = jax.devices()[:n_cores]
    assert len(devices) == n_cores
    mesh = Mesh(_np.asarray(devices), ("core",))
    in_specs = (PartitionSpec("core"),) * (n_params + n_outs)
    out_specs = (PartitionSpec("core"),) * n_outs
    donate = tuple(range(n_params, n_params + n_outs))
    sharded = jax.jit(
        shard_map(
            _body, mesh=mesh, in_specs=in_specs, out_specs=out_specs, check_rep=False
        ),
        donate_argnums=donate,
        keep_unused=True,
    )

    def run(in_maps):
        concat_in = [
            _np.concatenate([_np.asarray(in_maps[c][nm]) for c in range(n_cores)], axis=0)
            for nm in in_names
        ]
        concat_zeros = [
            _np.zeros((n_cores * z.shape[0], *z.shape[1:]), z.dtype) for z in zero_outs
        ]
        out_arrs = sharded(*concat_in, *concat_zeros)
        out_arrs = [_np.asarray(o) for o in out_arrs]
        return [
            {
                nm: out_arrs[i].reshape(n_cores, *out_avals[i].shape)[c]
                for i, nm in enumerate(out_names)
            }
            for c in range(n_cores)
        ]

    return run


def _route(flat, Wr, br, k):
    logits = flat.astype(np.float64) @ Wr.astype(np.float64) + br.astype(np.float64)
    order = np.argsort(-logits, axis=1, kind="stable")
    return order[:, :k]


def _host_expert(xe, W1e, b1e, W2e, b2e):
    h = xe.astype(np.float64) @ W1e.astype(np.float64) + b1e.astype(np.float64)
    erf = np.vectorize(math.erf)
    h = 0.5 * h * (1.0 + erf(h / math.sqrt(2.0)))
    return h @ W2e.astype(np.float64) + b2e.astype(np.float64)


def _prepare(inputs):
    x = np.asarray(inputs["x"], np.float32)
    Wr = np.asarray(inputs["Wr"], np.float32)
    br = np.asarray(inputs["br"], np.float32)
    W1 = np.asarray(inputs["W1"], np.float32)
    b1 = np.asarray(inputs["b1"], np.float32)
    W2 = np.asarray(inputs["W2"], np.float32)
    b2 = np.asarray(inputs["b2"], np.float32)
    k = int(np.asarray(inputs["k"]))
    assert x.shape == (_B, _SEQ, _E), x.shape

    flat = x.reshape(_T, _E)
    topk = _route(flat, Wr, br, k)
    flatT = np.ascontiguousarray(flat.T)

    in_maps = []
    idx_list = []
    overflow = []
    for e in range(_NE):
        idx = np.nonzero((topk == e).any(axis=1))[0]
        if len(idx) > _CAP:
            overflow.append((e, idx[_CAP:]))
            idx = idx[:_CAP]
        idx_list.append(idx)
        xt = np.zeros((_E, _CAP), np.float32)
        xt[:, : len(idx)] = flatT[:, idx]
        in_maps.append(
            {
                "xt": xt,
                "w1": np.ascontiguousarray(W1[e]),
                "w2": np.ascontiguousarray(W2[e]),
                "b1p": np.ascontiguousarray(b1[e].reshape(_H // _P, _P).T),
                "b2p": np.ascontiguousarray((b2[e] / k).reshape(_E // _P, _P).T),
            }
        )
    return flat, k, in_maps, idx_list, overflow, (W1, b1, W2, b2)


def kernel(**inputs) -> np.ndarray:
    flat, k, in_maps, idx_list, overflow, wb = _prepare(inputs)
    if overflow:
        # recompute overflow rows fully on host (exact erf gelu)
        W1, b1, W2, b2 = wb
        extra = [(e, idx, _host_expert(flat[idx], W1[e], b1[e], W2[e], b2[e]) / k)
                 for e, idx in overflow]
    else:
        extra = []

    key = (float(1.0 / k),)
    if key not in _nc_cache:
        nc = _build_nc(1.0 / k)
        _nc_cache[key] = _make_runner(nc, _NCORES)
    run = _nc_cache[key]
    results = run(in_maps)

    out = np.zeros((_T, _E), np.float32)
    for e in range(_NE):
        yt = results[e]["yt"]
        n = len(idx_list[e])
        out[idx_list[e]] += yt[:, :n].T
    for e, idx, y in extra:
        out[idx] += y.astype(np.float32)
    return out.reshape(_B, _SEQ, _E)
